# revision 20
# baseline (speedup 1.0000x reference)
"""Multi-head attention (B=2, T=4096, D=512, H=8) on 8 Trainium2 cores.

Sharding: core i handles batch b=i//4, query rows q0=(i%4)*1024 .. q0+1024,
all 8 heads (full K/V of its batch computed on-core; no collectives).
Host pre-transposes x and weights (bf16) and rolls x along T per core so
each core's query block sits at columns 0:1024.

v3 pipeline (evacuation-bound design):
- v2 was Activation-engine bound (33.5M exps/core at 1/cycle/lane on ACT
  ~ 252us busy).  v3 splits the exp between ACT (native Exp -> fp8e4
  values) and DVE (Schraudolph bit-trick: bits = round(s*1.4427 + 55.0)
  as uint8 IS the fp8e4m3 encoding of ~exp(s/8); the uniform scale it
  introduces cancels in the softmax normalization since the rowsum is
  built from the same weights).  Chunk assignment is load-balanced at
  build time; K/Q/V drains fill the ring-WAR gaps between exp chunks.
- `at` weights are fp8e4 -> AV runs as fp8 DoubleRow with 256-deep
  contraction ([128,2,q] stationary x [128,2,65] moving per key-block):
  4x fewer PE cycles than bf16 AV.  V is projected in bf16 but STORED
  fp8 (the quantization noise averages out over 4096 keys); all 4 query
  blocks of a phase accumulate into one PSUM bank at 512B-aligned
  offsets and share one batched reciprocal_approx_fast [128,4].
- AV for phase ph runs compactly right after ph's chunks (PE fills its
  own exp-tail wait), so the wrk PSUM pool cycles freely for projection
  pieces; transposes/O-projections are deferred into the next phase as
  fillers.  Projections bf16; scores fp8e4 DoubleRow with zeroed second
  k-tile slot; bv folded into bo on the host, bq/bk folded into drains.
"""
import sys
sys.path.insert(0, "/opt/trn_rl_repo")

import numpy as np
import ml_dtypes
import concourse.bacc as bacc
import concourse.mybir as mybir
import concourse.tile as tile
from concourse.bass_utils import run_bass_kernel_spmd

F32 = mybir.dt.float32
F32R = mybir.dt.float32r
BF16 = mybir.dt.bfloat16
F8 = mybir.dt.float8e4
U8 = mybir.dt.uint8
AF = mybir.ActivationFunctionType
ADD = mybir.AluOpType.add
MULT = mybir.AluOpType.mult
DR = mybir.MatmulPerfMode.DoubleRow

B, T, C = 2, 4096, 512
H, DK = 8, 64
TQ = 1024          # queries per core
NP = 4             # head pairs
KT = T // 128      # 32 k-tiles
NB = KT // 2       # 16 double-row key blocks
CT = C // 128      # 4 contraction tiles
NPH = 2 * H        # 16 phases (head, q-half)

EXP_C1 = 1.4426950408889634   # 0.125 * log2(e) * 8
EXP_C2 = 55.0                 # centers the Schraudolph sawtooth for fp8e4m3

# per-phase score chunks (kt0, n_kt): 16x2 (1024-col ring tiles, depth 3)
CHUNKS = [(2 * c, 2) for c in range(16)]

_cache = {}


def _build():
    nc = bacc.Bacc("TRN2")
    xbT = nc.declare_dram_parameter("xbT", [C, T], BF16, isOutput=False)
    wqT = nc.declare_dram_parameter("wqT", [C, C], BF16, isOutput=False)
    wkT = nc.declare_dram_parameter("wkT", [C, C], BF16, isOutput=False)
    wvT = nc.declare_dram_parameter("wvT", [C, C], BF16, isOutput=False)
    woT = nc.declare_dram_parameter("woT", [C, C], BF16, isOutput=False)
    # bias[:, 0, p] = bq slice, bias[:, 1, p] = bk slice
    bias = nc.declare_dram_parameter("bias", [128, 2, NP], F32, isOutput=False)
    bof = nc.declare_dram_parameter("bof", [1, C], F32R, isOutput=False)
    ones1 = nc.declare_dram_parameter("ones1", [1, 128], F32R, isOutput=False)
    eye = nc.declare_dram_parameter("eye", [128, 128], BF16, isOutput=False)
    out = nc.declare_dram_parameter("out", [TQ, C], BF16, isOutput=True)

    # build-time static load balancer for the two PSUM-evacuation engines
    load = {"act": 0.0, "dve": 0.0}

    def pick(cols):
        ca = cols * 0.8333 + 190.0
        cd = (cols * 1.0417 + 130.0) * 1.05
        if load["act"] + ca <= load["dve"] + cd:
            load["act"] += ca
            return "act"
        load["dve"] += cd
        return "dve"

    with tile.TileContext(nc) as tc:
        with (
            tc.tile_pool(name="big", bufs=1) as bpool,
            tc.tile_pool(name="v2", bufs=2) as v2pool,
            tc.tile_pool(name="rc", bufs=4) as rpool,
            tc.tile_pool(name="ot", bufs=4) as opool,
            tc.tile_pool(name="ring", bufs=3, space="PSUM") as ring,
            tc.tile_pool(name="wrk", bufs=2, space="PSUM") as wrk,
        ):
            # ---- static SBUF tiles ----
            xT = bpool.tile([128, CT, T], BF16, tag="xT")        # 32KB/part
            woTs = bpool.tile([128, CT, C], BF16, tag="woT")     # 4KB
            biasS = bpool.tile([128, 2, NP], F32, tag="bias")
            onesO = bpool.tile([65, 128], F32R, tag="ones")
            boS = bpool.tile([65, C], F32R, tag="bo")
            # fp8 K^T/Q^T, double-buffered by pair parity; [:,1,:] stays 0
            kf8a = bpool.tile([128, 2, T], F8, tag="kf8a")
            kf8b = bpool.tile([128, 2, T], F8, tag="kf8b")
            qf8a = bpool.tile([128, 2, TQ], F8, tag="qf8a")
            qf8b = bpool.tile([128, 2, TQ], F8, tag="qf8b")
            kf8 = [kf8a, kf8b]
            qf8 = [qf8a, qf8b]
            # exp output (fp8e4 weights), double-buffered by phase parity
            at0 = bpool.tile([128, KT, 512], F8, tag="at0")      # 16KB
            at1 = bpool.tile([128, KT, 512], F8, tag="at1")      # 16KB
            at = [at0, at1]
            # normalized AV, [q, d-pair]; double-buffered by pair parity
            avn0 = bpool.tile([128, 8, 128], BF16, tag="avn0")
            avn1 = bpool.tile([128, 8, 128], BF16, tag="avn1")
            avn = [avn0, avn1]
            acat = bpool.tile([128, NP, TQ], BF16, tag="acat")   # 8KB
            oacc = bpool.tile([128, 8, C], F32, tag="oacc")      # 16KB
            wkS = bpool.tile([128, CT, C], BF16, tag="wkS")
            wqS = bpool.tile([128, CT, C], BF16, tag="wqS")
            wvS = bpool.tile([128, CT, C], BF16, tag="wvS")
            eyeS = bpool.tile([128, 128], BF16, tag="eye")

            # ---- prologue DMAs ----
            xv = xbT.rearrange("(ct p) t -> p ct t", p=128)
            wkv = wkT.rearrange("(ct p) c -> p ct c", p=128)
            wqv = wqT.rearrange("(ct p) c -> p ct c", p=128)
            wvv = wvT.rearrange("(ct p) c -> p ct c", p=128)
            nc.sync.dma_start(wkS[:], wkv[:])
            nc.sync.dma_start(xT[:, :, 0:512], xv[:, :, 0:512])
            nc.sync.dma_start(wqS[:], wqv[:])
            nc.sync.dma_start(biasS[:], bias[:])
            nc.sync.dma_start(xT[:, :, 512:1024], xv[:, :, 512:1024])
            nc.sync.dma_start(wvS[:], wvv[:])
            for tch in range(1, 4):
                nc.sync.dma_start(
                    xT[:, :, tch * 1024:(tch + 1) * 1024],
                    xv[:, :, tch * 1024:(tch + 1) * 1024])
            nc.sync.dma_start(onesO[64:65, :],
                              ones1.rearrange("(o a) b -> o a b", o=1))
            nc.sync.dma_start(boS[64:65, :], bof.rearrange("(o a) b -> o a b", o=1))
            nc.sync.dma_start(woTs[:], woT.rearrange("(ct p) c -> p ct c", p=128))
            nc.sync.dma_start(eyeS[:], eye[:])
            dz = bpool.tile([64, 2, 512], F8, tag="dz")
            nc.vector.memset(dz[:], 0.0)
            nc.vector.memset(kf8[0][:, 1, 0:1536], 0.0)
            nc.vector.memset(qf8[0][:, 1, :], 0.0)
            nc.gpsimd.memset(kf8[0][:, 1, 1536:T], 0.0)
            nc.gpsimd.memset(kf8[1][:, 1, :], 0.0)
            nc.gpsimd.memset(qf8[1][:, 1, :], 0.0)

            # ---- balanced drain helpers ------------------------------------

            def drain_bias(dst, src, bcol, p, cols):
                if pick(cols) == "act":
                    nc.scalar.activation(dst, src, AF.Identity,
                                         bias=biasS[:, bcol, p:p + 1])
                else:
                    nc.vector.tensor_scalar_add(dst, src, biasS[:, bcol, p:p + 1])

            def drain_copy(dst, src, cols):
                if pick(cols) == "act":
                    nc.scalar.activation(dst, src, AF.Copy)
                else:
                    nc.vector.tensor_copy(dst, src)

            # ---- helper emitters -------------------------------------------

            def k_piece(p, piece):
                # K^T cols piece*512:(piece+1)*512 -> kf8[p%2][:, 0, ...]
                pp = wrk.tile([128, 512], F32, tag="wrk")
                for ct in range(CT):
                    nc.tensor.matmul(
                        pp[:], wkS[:, ct, p * 128:(p + 1) * 128],
                        xT[:, ct, piece * 512:(piece + 1) * 512],
                        start=(ct == 0), stop=(ct == CT - 1))
                drain_bias(kf8[p % 2][:, 0, piece * 512:(piece + 1) * 512],
                           pp[:], 1, p, 512)

            def q_piece(p, piece):
                pp = wrk.tile([128, 512], F32, tag="wrk")
                for ct in range(CT):
                    nc.tensor.matmul(
                        pp[:], wqS[:, ct, p * 128:(p + 1) * 128],
                        xT[:, ct, piece * 512:(piece + 1) * 512],
                        start=(ct == 0), stop=(ct == CT - 1))
                drain_bias(qf8[p % 2][:, 0, piece * 512:(piece + 1) * 512],
                           pp[:], 0, p, 512)

            def v_piece(v2p, pg, j):
                # V rows for k-tiles j, j+1 (bf16 matmuls, fp8 store)
                for jj in range(2):
                    kt = j + jj
                    pv = wrk.tile([128, 512], F32, tag="wrk")
                    for ct in range(CT):
                        nc.tensor.matmul(
                            pv[:, 0:256],
                            xT[:, ct, kt * 128:(kt + 1) * 128],
                            wvS[:, ct, pg * 256:(pg + 1) * 256],
                            start=(ct == 0), stop=(ct == CT - 1))
                    drain_copy(v2p[:, kt // 2, kt % 2, :, 0:64],
                               pv[:, 0:256].rearrange("p (h b) -> p h b", b=64),
                               256)

            def new_v2p():
                v2p = v2pool.tile([128, NB, 2, 4, 65], F8, tag="v2p")
                nc.gpsimd.memset(v2p[:, :, :, :, 64], 1.0)
                return v2p

            def exp_chunk(ph, kt0, n, ring_t):
                src = ring_t[:, 0:512 * n].rearrange("p (a b) -> p a b", b=512)
                dst = at[ph % 2][:, kt0:kt0 + n, :]
                if pick(512 * n) == "act":
                    nc.scalar.activation(dst, src, AF.Exp, scale=0.125)
                else:
                    nc.vector.tensor_scalar(dst.bitcast(U8), src,
                                            EXP_C1, EXP_C2, MULT, ADD)

            def av_mm_row(ph, b, v2p, av_t):
                # AV key-block b (kts 2b, 2b+1) for all 4 query blocks
                h = ph // 2
                for qb in range(4):
                    nc.tensor.matmul(
                        av_t[:, qb * 128:qb * 128 + 65],
                        at[ph % 2][:, 2 * b:2 * b + 2,
                                   qb * 128:(qb + 1) * 128],
                        v2p[:, b, :, h % 4, :],
                        start=(b == 0), stop=(b == NB - 1), perf_mode=DR)

            def av_finish(ph, av_t):
                # batched reciprocal + 4 normalizes; runs as the FIRST thunk
                # of phase ph+1 so the DVE never waits on the AV matmuls.
                h, half = ph // 2, ph % 2
                d0 = (h % 2) * 64
                pb = (h // 2) % 2
                p = h // 2
                rec = rpool.tile([128, 4, 1], F32, tag="rec")
                nc.vector.reciprocal_approx_fast(
                    rec[:],
                    av_t[:].rearrange("p (a b) -> p a b", b=128)[:, :, 64:65])
                load["dve"] += 140.0
                for qb in range(4):
                    qbg = half * 4 + qb
                    nc.vector.tensor_scalar(
                        avn[pb][:, qbg, d0:d0 + 64],
                        av_t[:, qb * 128:qb * 128 + 64],
                        rec[:, qb, :], None, MULT)
                    load["dve"] += 200.0
                    if h % 2 == 1:
                        if p == NP - 1:
                            # runs during phase 15: transpose + O inline
                            tr_o(p, qbg)
                        else:
                            state["pending"].append(
                                lambda p=p, qbg=qbg: tr_o(p, qbg))

            def o_piece(qt, po=None):
                if po is None:
                    po = wrk.tile([128, 512], F32, tag="wrk", name="po")
                nc.tensor.matmul(
                    po[:], acat[:, 3, qt * 128:(qt + 1) * 128],
                    woTs[:, 3, :], start=True, stop=True)
                ot = opool.tile([128, 512], BF16, tag="ot")
                nc.vector.tensor_tensor(out=ot[:], in0=po[:],
                                        in1=oacc[:, qt, :], op=ADD)
                load["dve"] += 690.0
                nc.sync.dma_start(out[qt * 128:(qt + 1) * 128, :], ot[:])

            def o_partial(qt):
                po = wrk.tile([128, 512], F32, tag="wrk")
                for r in range(3):
                    nc.tensor.matmul(
                        po[:], acat[:, r, qt * 128:(qt + 1) * 128],
                        woTs[:, r, :], start=(r == 0), stop=False)
                nc.tensor.matmul(po[:], onesO[64:65, :], boS[64:65, :],
                                 start=False, stop=True)
                drain_copy(oacc[:, qt, :], po[:], 512)

            def tr_o(p, qbg):
                nc.sync.dma_start_transpose(
                    acat[:, p, qbg * 128:(qbg + 1) * 128],
                    avn[p % 2][:, qbg, :])
                if p == NP - 1:
                    o_piece(qbg)

            # ---- prologue compute: pair-0 K/Q on ring slots ----------------

            def ring_kq(groups):
                """groups: list of ('k'|'q', p, piece). One ring tile, one
                matmul group per bank, batched drains per contiguous run."""
                rt = ring.tile([128, 1024], F32, tag="ring")
                for g, (kind, p, piece) in enumerate(groups):
                    w = wkS if kind == "k" else wqS
                    for ct in range(CT):
                        nc.tensor.matmul(
                            rt[:, g * 512:(g + 1) * 512],
                            w[:, ct, p * 128:(p + 1) * 128],
                            xT[:, ct, piece * 512:(piece + 1) * 512],
                            start=(ct == 0), stop=(ct == CT - 1))
                g = 0
                while g < len(groups):
                    kind, p, piece = groups[g]
                    g2 = g
                    while (g2 + 1 < len(groups)
                           and groups[g2 + 1][0] == kind
                           and groups[g2 + 1][2] == groups[g2][2] + 1):
                        g2 += 1
                    dst = kf8[p % 2] if kind == "k" else qf8[p % 2]
                    bcol = 1 if kind == "k" else 0
                    drain_bias(
                        dst[:, 0, piece * 512:piece * 512 + (g2 - g + 1) * 512],
                        rt[:, g * 512:(g2 + 1) * 512], bcol, p,
                        (g2 - g + 1) * 512)
                    g = g2 + 1

            # PE p-state warm-up on zeros while x loads
            wup = wrk.tile([128, 512], F32, tag="wrk")
            for i in range(10):
                nc.tensor.matmul(wup[:], dz[:, :, 0:128], dz[:],
                                 start=True, stop=True, perf_mode=DR,
                                 tile_position=(0, 0))
            ring_kq([("k", 0, 0)])
            q_piece(0, 0)
            v2p_cur = new_v2p()

            # ---- main pipeline over 16 phases ----
            state = {"v2p": v2p_cur, "v2p_next": None, "pending": [],
                     "av_t": None}

            def phase_background(ph):
                """Thunks to interleave into phase ph's chunk stream."""
                thunks = []
                h, half = ph // 2, ph % 2
                p = h // 2
                if state["av_t"] is not None:
                    av_t_prev = state["av_t"]
                    state["av_t"] = None
                    thunks.append(lambda ph=ph, t=av_t_prev:
                                  av_finish(ph - 1, t))
                pend, state["pending"] = state["pending"], []
                thunks.extend(pend)
                # pair-0 K/Q pieces MUST lead phase 0: its own chunks read
                # them piece-by-piece (program order is the only ordering)
                if ph == 0:
                    thunks.append(lambda: ring_kq([("q", 0, 1), ("k", 0, 1)]))
                    thunks.append(lambda: ring_kq([("k", 0, 2), ("k", 0, 3)]))
                    thunks.append(lambda: ring_kq([("k", 0, 4), ("k", 0, 5)]))
                    thunks.append(lambda: k_piece(0, 6))
                    thunks.append(lambda: k_piece(0, 7))
                # projection prep for pair p+1, spread over all 4 slots
                slot = ph % 4
                if p + 1 < NP:
                    if slot == 0:
                        for piece in range(3):
                            thunks.append(lambda p=p, piece=piece:
                                          k_piece(p + 1, piece))
                    elif slot == 1:
                        for piece in range(3, 6):
                            thunks.append(lambda p=p, piece=piece:
                                          k_piece(p + 1, piece))
                    elif slot == 2:
                        for piece in range(6, 8):
                            thunks.append(lambda p=p, piece=piece:
                                          k_piece(p + 1, piece))
                        thunks.append(lambda p=p: q_piece(p + 1, 0))
                    elif slot == 3:
                        thunks.append(lambda p=p: q_piece(p + 1, 1))
                if ph == 0:
                    for j in range(0, KT, 2):
                        thunks.append(lambda j=j: v_piece(state["v2p"], 0, j))
                if 4 <= ph <= 7:
                    if ph == 4:
                        def mkv():
                            state["v2p_next"] = new_v2p()
                        thunks.append(mkv)
                    for j in range((ph - 4) * 8, (ph - 4) * 8 + 8, 2):
                        thunks.append(lambda j=j: v_piece(state["v2p_next"],
                                                          1, j))
                if ph == 13:
                    for qt in range(4):
                        thunks.append(lambda qt=qt: o_partial(qt))
                if ph == 14:
                    for qt in range(4, 8):
                        thunks.append(lambda qt=qt: o_partial(qt))
                return thunks

            for ph in range(NPH):
                h, half = ph // 2, ph % 2
                if ph == 8:
                    state["v2p"] = state["v2p_next"]
                d0 = (h % 2) * 64
                kcur, qcur = kf8[h // 2 % 2], qf8[h // 2 % 2]
                bg = phase_background(ph)
                bgi = 0
                ncH = len(CHUNKS)
                for ci, (kt0, n) in enumerate(CHUNKS):
                    ring_t = ring.tile([128, 1024], F32, tag="ring")
                    for jj in range(n):
                        kt = kt0 + jj
                        nc.tensor.matmul(
                            ring_t[:, jj * 512:(jj + 1) * 512],
                            kcur[d0:d0 + 64, :, kt * 128:(kt + 1) * 128],
                            qcur[d0:d0 + 64, :, half * 512:(half + 1) * 512],
                            start=True, stop=True, perf_mode=DR,
                            tile_position=(d0, 0))
                    exp_chunk(ph, kt0, n, ring_t)
                    n_bg = (min(len(bg), (len(bg) * (ci + 1) + ncH - 1) // ncH)
                            - min(len(bg), (len(bg) * ci + ncH - 1) // ncH))
                    for _ in range(n_bg):
                        bg[bgi]()
                        bgi += 1
                assert bgi == len(bg)
                state["av_t"] = wrk.tile([128, 512], F32, tag="wrk",
                                         name="av_t")
                for qb in range(4):
                    for b in range(NB):
                        nc.tensor.matmul(
                            state["av_t"][:, qb * 128:qb * 128 + 65],
                            at[ph % 2][:, 2 * b:2 * b + 2,
                                       qb * 128:(qb + 1) * 128],
                            state["v2p"][:, b, :, (ph // 2) % 4, :],
                            start=(b == 0), stop=(b == NB - 1), perf_mode=DR)

            # ---- epilogue: AV(15) finish, per-qb pipelined with PE-mode
            # transposes (no 2.4us DMA-transpose latency at the tail) ----
            avE = state["av_t"]
            recE = rpool.tile([128, 4, 1], F32, tag="rec")
            nc.vector.reciprocal_approx_fast(
                recE[:],
                avE[:].rearrange("p (a b) -> p a b", b=128)[:, :, 64:65])
            for qb in range(4):
                qbg = 4 + qb
                nc.vector.tensor_scalar(
                    avn[1][:, qbg, 64:128],
                    avE[:, qb * 128:qb * 128 + 64],
                    recE[:, qb, :], None, MULT)
                tr_o(3, qbg)

    nc.compile()
    return nc


def _prep_inputs(x, Wq, bq, Wk, bk, Wv, bv, Wo, bo):
    bf = ml_dtypes.bfloat16
    wqT = np.ascontiguousarray(Wq.T).astype(bf)
    wkT = np.ascontiguousarray(Wk.T).astype(bf)
    wvT = np.ascontiguousarray(Wv.T).astype(bf)
    woT = np.ascontiguousarray(Wo.T).astype(bf)
    bias = np.stack([
        bq.reshape(NP, 128).T,
        bk.reshape(NP, 128).T,
    ], axis=1).astype(np.float32)          # [128, 2, NP]
    bias = np.ascontiguousarray(bias)
    bof = np.ascontiguousarray(
        (bo.astype(np.float64) + bv.astype(np.float64) @ Wo.astype(np.float64).T)
        .reshape(1, C)).astype(np.float32)
    ones1 = np.ones((1, 128), np.float32)
    eye = np.eye(128, dtype=bf)
    in_maps = []
    for i in range(8):
        b, q0 = i // 4, (i % 4) * TQ
        xbT = np.ascontiguousarray(np.roll(x[b].T, -q0, axis=1)).astype(bf)
        in_maps.append({
            "xbT": xbT, "wqT": wqT, "wkT": wkT, "wvT": wvT, "woT": woT,
            "bias": bias, "bof": bof, "ones1": ones1, "eye": eye,
        })
    return in_maps


def kernel(x, Wq, bq, Wk, bk, Wv, bv, Wo, bo):
    x = np.asarray(x, np.float32)
    args = [np.asarray(a, np.float32) for a in
            (Wq, bq, Wk, bk, Wv, bv, Wo, bo)]
    if "nc" not in _cache:
        _cache["nc"] = _build()
    nc = _cache["nc"]
    in_maps = _prep_inputs(x, *args)
    res = run_bass_kernel_spmd(nc, in_maps, list(range(8)))
    outf = np.empty((B, T, C), np.float32)
    for i in range(8):
        b, q0 = i // 4, (i % 4) * TQ
        outf[b, q0:q0 + TQ, :] = res.results[i]["out"].astype(np.float32)
    return outf


# revision 21
# speedup vs baseline: 1.0001x; 1.0001x over previous
"""Multi-head attention (B=2, T=4096, D=512, H=8) on 8 Trainium2 cores.

Sharding: core i handles batch b=i//4, query rows q0=(i%4)*1024 .. q0+1024,
all 8 heads (full K/V of its batch computed on-core; no collectives).
Host pre-transposes x and weights (bf16) and rolls x along T per core so
each core's query block sits at columns 0:1024.

v3 pipeline (evacuation-bound design):
- v2 was Activation-engine bound (33.5M exps/core at 1/cycle/lane on ACT
  ~ 252us busy).  v3 splits the exp between ACT (native Exp -> fp8e4
  values) and DVE (Schraudolph bit-trick: bits = round(s*1.4427 + 55.0)
  as uint8 IS the fp8e4m3 encoding of ~exp(s/8); the uniform scale it
  introduces cancels in the softmax normalization since the rowsum is
  built from the same weights).  Chunk assignment is load-balanced at
  build time; K/Q/V drains fill the ring-WAR gaps between exp chunks.
- `at` weights are fp8e4 -> AV runs as fp8 DoubleRow with 256-deep
  contraction ([128,2,q] stationary x [128,2,65] moving per key-block):
  4x fewer PE cycles than bf16 AV.  V is projected in bf16 but STORED
  fp8 (the quantization noise averages out over 4096 keys); all 4 query
  blocks of a phase accumulate into one PSUM bank at 512B-aligned
  offsets and share one batched reciprocal_approx_fast [128,4].
- AV for phase ph runs compactly right after ph's chunks (PE fills its
  own exp-tail wait), so the wrk PSUM pool cycles freely for projection
  pieces; transposes/O-projections are deferred into the next phase as
  fillers.  Projections bf16; scores fp8e4 DoubleRow with zeroed second
  k-tile slot; bv folded into bo on the host, bq/bk folded into drains.
"""
import sys
sys.path.insert(0, "/opt/trn_rl_repo")

import numpy as np
import ml_dtypes
import concourse.bacc as bacc
import concourse.mybir as mybir
import concourse.tile as tile
from concourse.bass_utils import run_bass_kernel_spmd

F32 = mybir.dt.float32
F32R = mybir.dt.float32r
BF16 = mybir.dt.bfloat16
F8 = mybir.dt.float8e4
U8 = mybir.dt.uint8
AF = mybir.ActivationFunctionType
ADD = mybir.AluOpType.add
MULT = mybir.AluOpType.mult
DR = mybir.MatmulPerfMode.DoubleRow

B, T, C = 2, 4096, 512
H, DK = 8, 64
TQ = 1024          # queries per core
NP = 4             # head pairs
KT = T // 128      # 32 k-tiles
NB = KT // 2       # 16 double-row key blocks
CT = C // 128      # 4 contraction tiles
NPH = 2 * H        # 16 phases (head, q-half)

EXP_C1 = 1.4426950408889634   # 0.125 * log2(e) * 8
EXP_C2 = 55.0                 # centers the Schraudolph sawtooth for fp8e4m3

# per-phase score chunks (kt0, n_kt): 16x2 (1024-col ring tiles, depth 3)
CHUNKS = [(2 * c, 2) for c in range(16)]

_cache = {}


def _build():
    nc = bacc.Bacc("TRN2")
    xbT = nc.declare_dram_parameter("xbT", [C, T], BF16, isOutput=False)
    wqT = nc.declare_dram_parameter("wqT", [C, C], BF16, isOutput=False)
    wkT = nc.declare_dram_parameter("wkT", [C, C], BF16, isOutput=False)
    wvT = nc.declare_dram_parameter("wvT", [C, C], BF16, isOutput=False)
    woT = nc.declare_dram_parameter("woT", [C, C], BF16, isOutput=False)
    # bias[:, 0, p] = bq slice, bias[:, 1, p] = bk slice
    bias = nc.declare_dram_parameter("bias", [128, 2, NP], F32, isOutput=False)
    bof = nc.declare_dram_parameter("bof", [1, C], F32R, isOutput=False)
    ones1 = nc.declare_dram_parameter("ones1", [1, 128], F32R, isOutput=False)
    eye = nc.declare_dram_parameter("eye", [128, 128], BF16, isOutput=False)
    out = nc.declare_dram_parameter("out", [TQ, C], BF16, isOutput=True)

    # build-time static load balancer for the two PSUM-evacuation engines
    load = {"act": 0.0, "dve": 0.0}

    def pick(cols):
        ca = cols * 0.8333 + 190.0
        cd = (cols * 1.0417 + 130.0) * 1.05
        if load["act"] + ca <= load["dve"] + cd:
            load["act"] += ca
            return "act"
        load["dve"] += cd
        return "dve"

    with tile.TileContext(nc) as tc:
        with (
            tc.tile_pool(name="big", bufs=1) as bpool,
            tc.tile_pool(name="v2", bufs=2) as v2pool,
            tc.tile_pool(name="rc", bufs=4) as rpool,
            tc.tile_pool(name="ot", bufs=4) as opool,
            tc.tile_pool(name="ring", bufs=3, space="PSUM") as ring,
            tc.tile_pool(name="wrk", bufs=2, space="PSUM") as wrk,
        ):
            # ---- static SBUF tiles ----
            xT = bpool.tile([128, CT, T], BF16, tag="xT")        # 32KB/part
            woTs = bpool.tile([128, CT, C], BF16, tag="woT")     # 4KB
            biasS = bpool.tile([128, 2, NP], F32, tag="bias")
            onesO = bpool.tile([65, 128], F32R, tag="ones")
            boS = bpool.tile([65, C], F32R, tag="bo")
            # fp8 K^T/Q^T, double-buffered by pair parity; [:,1,:] stays 0
            kf8a = bpool.tile([128, 2, T], F8, tag="kf8a")
            kf8b = bpool.tile([128, 2, T], F8, tag="kf8b")
            qf8a = bpool.tile([128, 2, TQ], F8, tag="qf8a")
            qf8b = bpool.tile([128, 2, TQ], F8, tag="qf8b")
            kf8 = [kf8a, kf8b]
            qf8 = [qf8a, qf8b]
            # exp output (fp8e4 weights), double-buffered by phase parity
            at0 = bpool.tile([128, KT, 512], F8, tag="at0")      # 16KB
            at1 = bpool.tile([128, KT, 512], F8, tag="at1")      # 16KB
            at = [at0, at1]
            # normalized AV, [q, d-pair]; double-buffered by pair parity
            avn0 = bpool.tile([128, 8, 128], BF16, tag="avn0")
            avn1 = bpool.tile([128, 8, 128], BF16, tag="avn1")
            avn = [avn0, avn1]
            acat = bpool.tile([128, NP, TQ], BF16, tag="acat")   # 8KB
            oacc = bpool.tile([128, 8, C], F32, tag="oacc")      # 16KB
            wkS = bpool.tile([128, CT, C], BF16, tag="wkS")
            wqS = bpool.tile([128, CT, C], BF16, tag="wqS")
            wvS = bpool.tile([128, CT, C], BF16, tag="wvS")
            eyeS = bpool.tile([128, 128], BF16, tag="eye")

            # ---- prologue DMAs ----
            xv = xbT.rearrange("(ct p) t -> p ct t", p=128)
            wkv = wkT.rearrange("(ct p) c -> p ct c", p=128)
            wqv = wqT.rearrange("(ct p) c -> p ct c", p=128)
            wvv = wvT.rearrange("(ct p) c -> p ct c", p=128)
            nc.sync.dma_start(wkS[:], wkv[:])
            nc.sync.dma_start(xT[:, :, 0:512], xv[:, :, 0:512])
            nc.sync.dma_start(wqS[:], wqv[:])
            nc.sync.dma_start(biasS[:], bias[:])
            nc.sync.dma_start(xT[:, :, 512:1024], xv[:, :, 512:1024])
            nc.sync.dma_start(wvS[:], wvv[:])
            for tch in range(1, 4):
                nc.sync.dma_start(
                    xT[:, :, tch * 1024:(tch + 1) * 1024],
                    xv[:, :, tch * 1024:(tch + 1) * 1024])
            nc.sync.dma_start(onesO[64:65, :],
                              ones1.rearrange("(o a) b -> o a b", o=1))
            nc.sync.dma_start(boS[64:65, :], bof.rearrange("(o a) b -> o a b", o=1))
            nc.sync.dma_start(woTs[:], woT.rearrange("(ct p) c -> p ct c", p=128))
            nc.sync.dma_start(eyeS[:], eye[:])
            dz = bpool.tile([64, 2, 512], F8, tag="dz")
            scrA = bpool.tile([64, 1], F8, tag="scrA")
            nc.vector.memset(dz[:], 0.0)
            # dummy exp pulls the ACT table load off the critical path
            nc.scalar.activation(scrA[:], dz[:, 0, 0:1], AF.Exp)
            nc.vector.memset(kf8[0][:, 1, 0:1536], 0.0)
            nc.vector.memset(qf8[0][:, 1, :], 0.0)
            nc.gpsimd.memset(kf8[0][:, 1, 1536:T], 0.0)
            nc.gpsimd.memset(kf8[1][:, 1, :], 0.0)
            nc.gpsimd.memset(qf8[1][:, 1, :], 0.0)

            # ---- balanced drain helpers ------------------------------------

            def drain_bias(dst, src, bcol, p, cols):
                if pick(cols) == "act":
                    nc.scalar.activation(dst, src, AF.Identity,
                                         bias=biasS[:, bcol, p:p + 1])
                else:
                    nc.vector.tensor_scalar_add(dst, src, biasS[:, bcol, p:p + 1])

            def drain_copy(dst, src, cols):
                if pick(cols) == "act":
                    nc.scalar.activation(dst, src, AF.Copy)
                else:
                    nc.vector.tensor_copy(dst, src)

            # ---- helper emitters -------------------------------------------

            def k_piece(p, piece):
                # K^T cols piece*512:(piece+1)*512 -> kf8[p%2][:, 0, ...]
                pp = wrk.tile([128, 512], F32, tag="wrk")
                for ct in range(CT):
                    nc.tensor.matmul(
                        pp[:], wkS[:, ct, p * 128:(p + 1) * 128],
                        xT[:, ct, piece * 512:(piece + 1) * 512],
                        start=(ct == 0), stop=(ct == CT - 1))
                drain_bias(kf8[p % 2][:, 0, piece * 512:(piece + 1) * 512],
                           pp[:], 1, p, 512)

            def q_piece(p, piece):
                pp = wrk.tile([128, 512], F32, tag="wrk")
                for ct in range(CT):
                    nc.tensor.matmul(
                        pp[:], wqS[:, ct, p * 128:(p + 1) * 128],
                        xT[:, ct, piece * 512:(piece + 1) * 512],
                        start=(ct == 0), stop=(ct == CT - 1))
                drain_bias(qf8[p % 2][:, 0, piece * 512:(piece + 1) * 512],
                           pp[:], 0, p, 512)

            def v_piece(v2p, pg, j):
                # V rows for k-tiles j, j+1 (bf16 matmuls, fp8 store)
                for jj in range(2):
                    kt = j + jj
                    pv = wrk.tile([128, 512], F32, tag="wrk")
                    for ct in range(CT):
                        nc.tensor.matmul(
                            pv[:, 0:256],
                            xT[:, ct, kt * 128:(kt + 1) * 128],
                            wvS[:, ct, pg * 256:(pg + 1) * 256],
                            start=(ct == 0), stop=(ct == CT - 1))
                    drain_copy(v2p[:, kt // 2, kt % 2, :, 0:64],
                               pv[:, 0:256].rearrange("p (h b) -> p h b", b=64),
                               256)

            def new_v2p():
                v2p = v2pool.tile([128, NB, 2, 4, 65], F8, tag="v2p")
                nc.gpsimd.memset(v2p[:, :, :, :, 64], 1.0)
                return v2p

            def exp_chunk(ph, kt0, n, ring_t):
                src = ring_t[:, 0:512 * n].rearrange("p (a b) -> p a b", b=512)
                dst = at[ph % 2][:, kt0:kt0 + n, :]
                if pick(512 * n) == "act":
                    nc.scalar.activation(dst, src, AF.Exp, scale=0.125)
                else:
                    nc.vector.tensor_scalar(dst.bitcast(U8), src,
                                            EXP_C1, EXP_C2, MULT, ADD)

            def av_mm_row(ph, b, v2p, av_t):
                # AV key-block b (kts 2b, 2b+1) for all 4 query blocks
                h = ph // 2
                for qb in range(4):
                    nc.tensor.matmul(
                        av_t[:, qb * 128:qb * 128 + 65],
                        at[ph % 2][:, 2 * b:2 * b + 2,
                                   qb * 128:(qb + 1) * 128],
                        v2p[:, b, :, h % 4, :],
                        start=(b == 0), stop=(b == NB - 1), perf_mode=DR)

            def av_finish(ph, av_t):
                # batched reciprocal + 4 normalizes; runs as the FIRST thunk
                # of phase ph+1 so the DVE never waits on the AV matmuls.
                h, half = ph // 2, ph % 2
                d0 = (h % 2) * 64
                pb = (h // 2) % 2
                p = h // 2
                rec = rpool.tile([128, 4, 1], F32, tag="rec")
                nc.vector.reciprocal_approx_fast(
                    rec[:],
                    av_t[:].rearrange("p (a b) -> p a b", b=128)[:, :, 64:65])
                load["dve"] += 140.0
                for qb in range(4):
                    qbg = half * 4 + qb
                    nc.vector.tensor_scalar(
                        avn[pb][:, qbg, d0:d0 + 64],
                        av_t[:, qb * 128:qb * 128 + 64],
                        rec[:, qb, :], None, MULT)
                    load["dve"] += 200.0
                if h % 2 == 1:
                    for qb in range(4):
                        qbg = half * 4 + qb
                        if p == NP - 1:
                            # runs during phase 15: transpose + O inline
                            tr_o(p, qbg)
                        else:
                            state["pending"].append(
                                lambda p=p, qbg=qbg: tr_o(p, qbg))

            def o_piece(qt, po=None):
                if po is None:
                    po = wrk.tile([128, 512], F32, tag="wrk", name="po")
                nc.tensor.matmul(
                    po[:], acat[:, 3, qt * 128:(qt + 1) * 128],
                    woTs[:, 3, :], start=True, stop=True)
                ot = opool.tile([128, 512], BF16, tag="ot")
                nc.vector.tensor_tensor(out=ot[:], in0=po[:],
                                        in1=oacc[:, qt, :], op=ADD)
                load["dve"] += 690.0
                nc.sync.dma_start(out[qt * 128:(qt + 1) * 128, :], ot[:])

            def o_partial(qt):
                po = wrk.tile([128, 512], F32, tag="wrk")
                for r in range(3):
                    nc.tensor.matmul(
                        po[:], acat[:, r, qt * 128:(qt + 1) * 128],
                        woTs[:, r, :], start=(r == 0), stop=False)
                nc.tensor.matmul(po[:], onesO[64:65, :], boS[64:65, :],
                                 start=False, stop=True)
                drain_copy(oacc[:, qt, :], po[:], 512)

            def tr_o(p, qbg):
                nc.sync.dma_start_transpose(
                    acat[:, p, qbg * 128:(qbg + 1) * 128],
                    avn[p % 2][:, qbg, :])
                if p == NP - 1:
                    o_piece(qbg)

            # ---- prologue compute: pair-0 K/Q on ring slots ----------------

            def ring_kq(groups):
                """groups: list of ('k'|'q', p, piece). One ring tile, one
                matmul group per bank, batched drains per contiguous run."""
                rt = ring.tile([128, 1024], F32, tag="ring")
                for g, (kind, p, piece) in enumerate(groups):
                    w = wkS if kind == "k" else wqS
                    for ct in range(CT):
                        nc.tensor.matmul(
                            rt[:, g * 512:(g + 1) * 512],
                            w[:, ct, p * 128:(p + 1) * 128],
                            xT[:, ct, piece * 512:(piece + 1) * 512],
                            start=(ct == 0), stop=(ct == CT - 1))
                g = 0
                while g < len(groups):
                    kind, p, piece = groups[g]
                    g2 = g
                    while (g2 + 1 < len(groups)
                           and groups[g2 + 1][0] == kind
                           and groups[g2 + 1][2] == groups[g2][2] + 1):
                        g2 += 1
                    dst = kf8[p % 2] if kind == "k" else qf8[p % 2]
                    bcol = 1 if kind == "k" else 0
                    drain_bias(
                        dst[:, 0, piece * 512:piece * 512 + (g2 - g + 1) * 512],
                        rt[:, g * 512:(g2 + 1) * 512], bcol, p,
                        (g2 - g + 1) * 512)
                    g = g2 + 1

            # PE p-state warm-up on zeros while x loads
            wup = wrk.tile([128, 512], F32, tag="wrk")
            for i in range(10):
                nc.tensor.matmul(wup[:], dz[:, :, 0:128], dz[:],
                                 start=True, stop=True, perf_mode=DR,
                                 tile_position=(0, 0))
            ring_kq([("k", 0, 0)])
            q_piece(0, 0)
            v2p_cur = new_v2p()

            # ---- main pipeline over 16 phases ----
            state = {"v2p": v2p_cur, "v2p_next": None, "pending": [],
                     "av_t": None}

            def phase_background(ph):
                """Thunks to interleave into phase ph's chunk stream."""
                thunks = []
                h, half = ph // 2, ph % 2
                p = h // 2
                pend, state["pending"] = state["pending"], []
                thunks.extend(pend[:2])
                if state["av_t"] is not None:
                    av_t_prev = state["av_t"]
                    state["av_t"] = None
                    thunks.append(lambda ph=ph, t=av_t_prev:
                                  av_finish(ph - 1, t))
                thunks.extend(pend[2:])
                # pair-0 K/Q pieces MUST lead phase 0: its own chunks read
                # them piece-by-piece (program order is the only ordering)
                if ph == 0:
                    thunks.append(lambda: ring_kq([("q", 0, 1), ("k", 0, 1)]))
                    thunks.append(lambda: ring_kq([("k", 0, 2), ("k", 0, 3)]))
                    thunks.append(lambda: ring_kq([("k", 0, 4), ("k", 0, 5)]))
                    thunks.append(lambda: k_piece(0, 6))
                    thunks.append(lambda: k_piece(0, 7))
                # projection prep for pair p+1, spread over all 4 slots
                slot = ph % 4
                if p + 1 < NP:
                    if slot == 0:
                        for piece in range(3):
                            thunks.append(lambda p=p, piece=piece:
                                          k_piece(p + 1, piece))
                    elif slot == 1:
                        for piece in range(3, 6):
                            thunks.append(lambda p=p, piece=piece:
                                          k_piece(p + 1, piece))
                    elif slot == 2:
                        for piece in range(6, 8):
                            thunks.append(lambda p=p, piece=piece:
                                          k_piece(p + 1, piece))
                        thunks.append(lambda p=p: q_piece(p + 1, 0))
                    elif slot == 3:
                        thunks.append(lambda p=p: q_piece(p + 1, 1))
                if ph == 0:
                    for j in range(0, KT, 2):
                        thunks.append(lambda j=j: v_piece(state["v2p"], 0, j))
                if 4 <= ph <= 7:
                    if ph == 4:
                        def mkv():
                            state["v2p_next"] = new_v2p()
                        thunks.append(mkv)
                    for j in range((ph - 4) * 8, (ph - 4) * 8 + 8, 2):
                        thunks.append(lambda j=j: v_piece(state["v2p_next"],
                                                          1, j))
                if ph == 13:
                    for qt in range(4):
                        thunks.append(lambda qt=qt: o_partial(qt))
                if ph == 14:
                    for qt in range(4, 8):
                        thunks.append(lambda qt=qt: o_partial(qt))
                return thunks

            for ph in range(NPH):
                h, half = ph // 2, ph % 2
                if ph == 8:
                    state["v2p"] = state["v2p_next"]
                d0 = (h % 2) * 64
                kcur, qcur = kf8[h // 2 % 2], qf8[h // 2 % 2]
                bg = phase_background(ph)
                bgi = 0
                ncH = len(CHUNKS)
                for ci, (kt0, n) in enumerate(CHUNKS):
                    ring_t = ring.tile([128, 1024], F32, tag="ring")
                    for jj in range(n):
                        kt = kt0 + jj
                        nc.tensor.matmul(
                            ring_t[:, jj * 512:(jj + 1) * 512],
                            kcur[d0:d0 + 64, :, kt * 128:(kt + 1) * 128],
                            qcur[d0:d0 + 64, :, half * 512:(half + 1) * 512],
                            start=True, stop=True, perf_mode=DR,
                            tile_position=(d0, 0))
                    exp_chunk(ph, kt0, n, ring_t)
                    n_bg = (min(len(bg), (len(bg) * (ci + 1) + ncH - 1) // ncH)
                            - min(len(bg), (len(bg) * ci + ncH - 1) // ncH))
                    for _ in range(n_bg):
                        bg[bgi]()
                        bgi += 1
                assert bgi == len(bg)
                state["av_t"] = wrk.tile([128, 512], F32, tag="wrk",
                                         name="av_t")
                for qb in range(4):
                    for b in range(NB):
                        nc.tensor.matmul(
                            state["av_t"][:, qb * 128:qb * 128 + 65],
                            at[ph % 2][:, 2 * b:2 * b + 2,
                                       qb * 128:(qb + 1) * 128],
                            state["v2p"][:, b, :, (ph // 2) % 4, :],
                            start=(b == 0), stop=(b == NB - 1), perf_mode=DR)

            # ---- epilogue: AV(15) finish, per-qb pipelined with PE-mode
            # transposes (no 2.4us DMA-transpose latency at the tail) ----
            avE = state["av_t"]
            recE = rpool.tile([128, 4, 1], F32, tag="rec")
            nc.vector.reciprocal_approx_fast(
                recE[:],
                avE[:].rearrange("p (a b) -> p a b", b=128)[:, :, 64:65])
            for qb in range(4):
                qbg = 4 + qb
                nc.vector.tensor_scalar(
                    avn[1][:, qbg, 64:128],
                    avE[:, qb * 128:qb * 128 + 64],
                    recE[:, qb, :], None, MULT)
            epR = [ring.tile([128, 1024], F32, tag="ring", name="epR0"),
                   ring.tile([128, 1024], F32, tag="ring", name="epR1")]
            for qb in range(4):
                qbg = 4 + qb
                trp = wrk.tile([128, 128], BF16, tag="wrk", name="trp")
                nc.tensor.transpose(trp[:], avn[1][:, qbg, :], eyeS[:])
                nc.vector.tensor_copy(acat[:, 3, qbg * 128:(qbg + 1) * 128],
                                      trp[:])
                o_piece(qbg, po=epR[qb // 2][:, (qb % 2) * 512:
                                             (qb % 2) * 512 + 512])

    nc.compile()
    return nc


def _prep_inputs(x, Wq, bq, Wk, bk, Wv, bv, Wo, bo):
    bf = ml_dtypes.bfloat16
    wqT = np.ascontiguousarray(Wq.T).astype(bf)
    wkT = np.ascontiguousarray(Wk.T).astype(bf)
    wvT = np.ascontiguousarray(Wv.T).astype(bf)
    woT = np.ascontiguousarray(Wo.T).astype(bf)
    bias = np.stack([
        bq.reshape(NP, 128).T,
        bk.reshape(NP, 128).T,
    ], axis=1).astype(np.float32)          # [128, 2, NP]
    bias = np.ascontiguousarray(bias)
    bof = np.ascontiguousarray(
        (bo.astype(np.float64) + bv.astype(np.float64) @ Wo.astype(np.float64).T)
        .reshape(1, C)).astype(np.float32)
    ones1 = np.ones((1, 128), np.float32)
    eye = np.eye(128, dtype=bf)
    in_maps = []
    for i in range(8):
        b, q0 = i // 4, (i % 4) * TQ
        xbT = np.ascontiguousarray(np.roll(x[b].T, -q0, axis=1)).astype(bf)
        in_maps.append({
            "xbT": xbT, "wqT": wqT, "wkT": wkT, "wvT": wvT, "woT": woT,
            "bias": bias, "bof": bof, "ones1": ones1, "eye": eye,
        })
    return in_maps


def kernel(x, Wq, bq, Wk, bk, Wv, bv, Wo, bo):
    x = np.asarray(x, np.float32)
    args = [np.asarray(a, np.float32) for a in
            (Wq, bq, Wk, bk, Wv, bv, Wo, bo)]
    if "nc" not in _cache:
        _cache["nc"] = _build()
    nc = _cache["nc"]
    in_maps = _prep_inputs(x, *args)
    res = run_bass_kernel_spmd(nc, in_maps, list(range(8)))
    outf = np.empty((B, T, C), np.float32)
    for i in range(8):
        b, q0 = i // 4, (i % 4) * TQ
        outf[b, q0:q0 + TQ, :] = res.results[i]["out"].astype(np.float32)
    return outf


# revision 22
# speedup vs baseline: 1.0104x; 1.0103x over previous
"""Multi-head attention (B=2, T=4096, D=512, H=8) on 8 Trainium2 cores.

Sharding: core i handles batch b=i//4, query rows q0=(i%4)*1024 .. q0+1024,
all 8 heads (full K/V of its batch computed on-core; no collectives).
Host pre-transposes x and weights (bf16) and rolls x along T per core so
each core's query block sits at columns 0:1024.

v3 pipeline (evacuation-bound design):
- v2 was Activation-engine bound (33.5M exps/core at 1/cycle/lane on ACT
  ~ 252us busy).  v3 splits the exp between ACT (native Exp -> fp8e4
  values) and DVE (Schraudolph bit-trick: bits = round(s*1.4427 + 55.0)
  as uint8 IS the fp8e4m3 encoding of ~exp(s/8); the uniform scale it
  introduces cancels in the softmax normalization since the rowsum is
  built from the same weights).  Chunk assignment is load-balanced at
  build time; K/Q/V drains fill the ring-WAR gaps between exp chunks.
- `at` weights are fp8e4 -> AV runs as fp8 DoubleRow with 256-deep
  contraction ([128,2,q] stationary x [128,2,65] moving per key-block):
  4x fewer PE cycles than bf16 AV.  V is projected in bf16 but STORED
  fp8 (the quantization noise averages out over 4096 keys); all 4 query
  blocks of a phase accumulate into one PSUM bank at 512B-aligned
  offsets and share one batched reciprocal_approx_fast [128,4].
- AV for phase ph runs compactly right after ph's chunks (PE fills its
  own exp-tail wait), so the wrk PSUM pool cycles freely for projection
  pieces; transposes/O-projections are deferred into the next phase as
  fillers.  Projections bf16; scores fp8e4 DoubleRow with zeroed second
  k-tile slot; bv folded into bo on the host, bq/bk folded into drains.
"""
import sys
sys.path.insert(0, "/opt/trn_rl_repo")

import numpy as np
import ml_dtypes
import concourse.bacc as bacc
import concourse.mybir as mybir
import concourse.tile as tile
from concourse.bass_utils import run_bass_kernel_spmd

F32 = mybir.dt.float32
F32R = mybir.dt.float32r
BF16 = mybir.dt.bfloat16
F8 = mybir.dt.float8e4
U8 = mybir.dt.uint8
AF = mybir.ActivationFunctionType
ADD = mybir.AluOpType.add
MULT = mybir.AluOpType.mult
DR = mybir.MatmulPerfMode.DoubleRow

B, T, C = 2, 4096, 512
H, DK = 8, 64
TQ = 1024          # queries per core
NP = 4             # head pairs
KT = T // 128      # 32 k-tiles
NB = KT // 2       # 16 double-row key blocks
CT = C // 128      # 4 contraction tiles
NPH = 2 * H        # 16 phases (head, q-half)

EXP_C1 = 1.4426950408889634   # 0.125 * log2(e) * 8
EXP_C2 = 55.0                 # centers the Schraudolph sawtooth for fp8e4m3

# per-phase score chunks (kt0, n_kt): 16x2 (1024-col ring tiles, depth 3)
CHUNKS = [(2 * c, 2) for c in range(16)]

_cache = {}


def _build():
    nc = bacc.Bacc("TRN2")
    xbT = nc.declare_dram_parameter("xbT", [C, T], BF16, isOutput=False)
    wqT = nc.declare_dram_parameter("wqT", [C, C], BF16, isOutput=False)
    wkT = nc.declare_dram_parameter("wkT", [C, C], BF16, isOutput=False)
    wvT = nc.declare_dram_parameter("wvT", [C, C], BF16, isOutput=False)
    woT = nc.declare_dram_parameter("woT", [C, C], BF16, isOutput=False)
    # bias[:, 0, p] = bq slice, bias[:, 1, p] = bk slice
    bias = nc.declare_dram_parameter("bias", [128, 2, NP], F32, isOutput=False)
    bof = nc.declare_dram_parameter("bof", [1, C], F32R, isOutput=False)
    ones1 = nc.declare_dram_parameter("ones1", [1, 128], F32R, isOutput=False)
    eye = nc.declare_dram_parameter("eye", [128, 128], BF16, isOutput=False)
    out = nc.declare_dram_parameter("out", [TQ, C], BF16, isOutput=True)

    # build-time static load balancer for the two PSUM-evacuation engines
    load = {"act": 0.0, "dve": 0.0}

    def pick(cols):
        ca = cols * 0.8333 + 190.0
        cd = (cols * 1.0417 + 130.0) * 1.05
        if load["act"] + ca <= load["dve"] + cd:
            load["act"] += ca
            return "act"
        load["dve"] += cd
        return "dve"

    with tile.TileContext(nc) as tc:
        with (
            tc.tile_pool(name="big", bufs=1) as bpool,
            tc.tile_pool(name="v2", bufs=2) as v2pool,
            tc.tile_pool(name="rc", bufs=4) as rpool,
            tc.tile_pool(name="ot", bufs=4) as opool,
            tc.tile_pool(name="ring", bufs=3, space="PSUM") as ring,
            tc.tile_pool(name="wrk", bufs=2, space="PSUM") as wrk,
        ):
            # ---- static SBUF tiles ----
            xT = bpool.tile([128, CT, T], BF16, tag="xT")        # 32KB/part
            woTs = bpool.tile([128, CT, C], BF16, tag="woT")     # 4KB
            biasS = bpool.tile([128, 2, NP], F32, tag="bias")
            onesO = bpool.tile([65, 128], F32R, tag="ones")
            boS = bpool.tile([65, C], F32R, tag="bo")
            # fp8 K^T/Q^T, double-buffered by pair parity; [:,1,:] stays 0
            kf8a = bpool.tile([128, 2, T], F8, tag="kf8a")
            kf8b = bpool.tile([128, 2, T], F8, tag="kf8b")
            qf8a = bpool.tile([128, 2, TQ], F8, tag="qf8a")
            qf8b = bpool.tile([128, 2, TQ], F8, tag="qf8b")
            kf8 = [kf8a, kf8b]
            qf8 = [qf8a, qf8b]
            # exp output (fp8e4 weights), double-buffered by phase parity
            at0 = bpool.tile([128, KT, 512], F8, tag="at0")      # 16KB
            at1 = bpool.tile([128, KT, 512], F8, tag="at1")      # 16KB
            at = [at0, at1]
            # normalized AV, [q, d-pair]; double-buffered by pair parity
            avn0 = bpool.tile([128, 8, 128], BF16, tag="avn0")
            avn1 = bpool.tile([128, 8, 128], BF16, tag="avn1")
            avn = [avn0, avn1]
            acat = bpool.tile([128, NP, TQ], BF16, tag="acat")   # 8KB
            oacc = bpool.tile([128, 8, C], F32, tag="oacc")      # 16KB
            wkS = bpool.tile([128, CT, C], BF16, tag="wkS")
            wqS = bpool.tile([128, CT, C], BF16, tag="wqS")
            wvS = bpool.tile([128, CT, C], BF16, tag="wvS")
            eyeS = bpool.tile([128, 128], BF16, tag="eye")

            # ---- prologue DMAs ----
            xv = xbT.rearrange("(ct p) t -> p ct t", p=128)
            wkv = wkT.rearrange("(ct p) c -> p ct c", p=128)
            wqv = wqT.rearrange("(ct p) c -> p ct c", p=128)
            wvv = wvT.rearrange("(ct p) c -> p ct c", p=128)
            nc.sync.dma_start(wkS[:], wkv[:])
            nc.sync.dma_start(xT[:, :, 0:512], xv[:, :, 0:512])
            nc.sync.dma_start(wqS[:], wqv[:])
            nc.sync.dma_start(biasS[:], bias[:])
            nc.sync.dma_start(xT[:, :, 512:1024], xv[:, :, 512:1024])
            nc.sync.dma_start(wvS[:], wvv[:])
            for tch in range(1, 4):
                nc.sync.dma_start(
                    xT[:, :, tch * 1024:(tch + 1) * 1024],
                    xv[:, :, tch * 1024:(tch + 1) * 1024])
            nc.sync.dma_start(onesO[64:65, :],
                              ones1.rearrange("(o a) b -> o a b", o=1))
            nc.sync.dma_start(boS[64:65, :], bof.rearrange("(o a) b -> o a b", o=1))
            nc.sync.dma_start(woTs[:], woT.rearrange("(ct p) c -> p ct c", p=128))
            nc.sync.dma_start(eyeS[:], eye[:])
            dz = bpool.tile([64, 2, 512], F8, tag="dz")
            scrA = bpool.tile([64, 1], F8, tag="scrA")
            nc.vector.memset(dz[:], 0.0)
            # dummy exp pulls the ACT table load off the critical path
            nc.scalar.activation(scrA[:], dz[:, 0, 0:1], AF.Exp)
            nc.gpsimd.memset(qf8[0][:, 1, :], 0.0)
            nc.gpsimd.memset(kf8[0][:, 1, :], 0.0)
            nc.gpsimd.memset(kf8[1][:, 1, :], 0.0)
            nc.gpsimd.memset(qf8[1][:, 1, :], 0.0)

            # ---- balanced drain helpers ------------------------------------

            def drain_bias(dst, src, bcol, p, cols):
                if pick(cols) == "act":
                    nc.scalar.activation(dst, src, AF.Identity,
                                         bias=biasS[:, bcol, p:p + 1])
                else:
                    nc.vector.tensor_scalar_add(dst, src, biasS[:, bcol, p:p + 1])

            def drain_copy(dst, src, cols):
                if pick(cols) == "act":
                    nc.scalar.activation(dst, src, AF.Copy)
                else:
                    nc.vector.tensor_copy(dst, src)

            # ---- helper emitters -------------------------------------------

            def k_piece(p, piece):
                # K^T cols piece*512:(piece+1)*512 -> kf8[p%2][:, 0, ...]
                pp = wrk.tile([128, 512], F32, tag="wrk")
                for ct in range(CT):
                    nc.tensor.matmul(
                        pp[:], wkS[:, ct, p * 128:(p + 1) * 128],
                        xT[:, ct, piece * 512:(piece + 1) * 512],
                        start=(ct == 0), stop=(ct == CT - 1))
                drain_bias(kf8[p % 2][:, 0, piece * 512:(piece + 1) * 512],
                           pp[:], 1, p, 512)

            def q_piece(p, piece):
                pp = wrk.tile([128, 512], F32, tag="wrk")
                for ct in range(CT):
                    nc.tensor.matmul(
                        pp[:], wqS[:, ct, p * 128:(p + 1) * 128],
                        xT[:, ct, piece * 512:(piece + 1) * 512],
                        start=(ct == 0), stop=(ct == CT - 1))
                drain_bias(qf8[p % 2][:, 0, piece * 512:(piece + 1) * 512],
                           pp[:], 0, p, 512)

            def v_piece(v2p, pg, j):
                # V rows for k-tiles j, j+1 (bf16 matmuls, fp8 store)
                for jj in range(2):
                    kt = j + jj
                    pv = wrk.tile([128, 512], F32, tag="wrk")
                    for ct in range(CT):
                        nc.tensor.matmul(
                            pv[:, 0:256],
                            xT[:, ct, kt * 128:(kt + 1) * 128],
                            wvS[:, ct, pg * 256:(pg + 1) * 256],
                            start=(ct == 0), stop=(ct == CT - 1))
                    drain_copy(v2p[:, kt // 2, kt % 2, :, 0:64],
                               pv[:, 0:256].rearrange("p (h b) -> p h b", b=64),
                               256)

            def new_v2p():
                v2p = v2pool.tile([128, NB, 2, 4, 65], F8, tag="v2p")
                nc.gpsimd.memset(v2p[:, :, :, :, 64], 1.0)
                return v2p

            def exp_chunk(ph, kt0, n, ring_t):
                src = ring_t[:, 0:512 * n].rearrange("p (a b) -> p a b", b=512)
                dst = at[ph % 2][:, kt0:kt0 + n, :]
                if pick(512 * n) == "act":
                    nc.scalar.activation(dst, src, AF.Exp, scale=0.125)
                else:
                    nc.vector.tensor_scalar(dst.bitcast(U8), src,
                                            EXP_C1, EXP_C2, MULT, ADD)

            def av_mm_row(ph, b, v2p, av_t):
                # AV key-block b (kts 2b, 2b+1) for all 4 query blocks
                h = ph // 2
                for qb in range(4):
                    nc.tensor.matmul(
                        av_t[:, qb * 128:qb * 128 + 65],
                        at[ph % 2][:, 2 * b:2 * b + 2,
                                   qb * 128:(qb + 1) * 128],
                        v2p[:, b, :, h % 4, :],
                        start=(b == 0), stop=(b == NB - 1), perf_mode=DR)

            def av_finish(ph, av_t):
                # batched reciprocal + 4 normalizes; runs as the FIRST thunk
                # of phase ph+1 so the DVE never waits on the AV matmuls.
                h, half = ph // 2, ph % 2
                d0 = (h % 2) * 64
                pb = (h // 2) % 2
                p = h // 2
                rec = rpool.tile([128, 4, 1], F32, tag="rec")
                nc.vector.reciprocal_approx_fast(
                    rec[:],
                    av_t[:].rearrange("p (a b) -> p a b", b=128)[:, :, 64:65])
                load["dve"] += 140.0
                for qb in range(4):
                    qbg = half * 4 + qb
                    nc.vector.tensor_scalar(
                        avn[pb][:, qbg, d0:d0 + 64],
                        av_t[:, qb * 128:qb * 128 + 64],
                        rec[:, qb, :], None, MULT)
                    load["dve"] += 200.0
                if h % 2 == 1:
                    for qb in range(4):
                        qbg = half * 4 + qb
                        if p == NP - 1:
                            # runs during phase 15: transpose now, O-piece
                            # queued so its DVE add doesn't block the
                            # remaining exp chunks in the FIFO
                            nc.sync.dma_start_transpose(
                                acat[:, 3, qbg * 128:(qbg + 1) * 128],
                                avn[1][:, qbg, :])
                            state["o_queue"].append(qbg)
                        else:
                            state["pending"].append(
                                lambda p=p, qbg=qbg: tr_o(p, qbg))

            def o_piece(qt, po=None):
                if po is None:
                    po = wrk.tile([128, 512], F32, tag="wrk", name="po")
                nc.tensor.matmul(
                    po[:], acat[:, 3, qt * 128:(qt + 1) * 128],
                    woTs[:, 3, :], start=True, stop=True)
                ot = opool.tile([128, 512], BF16, tag="ot")
                nc.vector.tensor_tensor(out=ot[:], in0=po[:],
                                        in1=oacc[:, qt, :], op=ADD)
                load["dve"] += 690.0
                nc.sync.dma_start(out[qt * 128:(qt + 1) * 128, :], ot[:])

            def o_partial(qt):
                po = wrk.tile([128, 512], F32, tag="wrk")
                for r in range(3):
                    nc.tensor.matmul(
                        po[:], acat[:, r, qt * 128:(qt + 1) * 128],
                        woTs[:, r, :], start=(r == 0), stop=False)
                nc.tensor.matmul(po[:], onesO[64:65, :], boS[64:65, :],
                                 start=False, stop=True)
                drain_copy(oacc[:, qt, :], po[:], 512)

            def tr_o(p, qbg):
                nc.sync.dma_start_transpose(
                    acat[:, p, qbg * 128:(qbg + 1) * 128],
                    avn[p % 2][:, qbg, :])
                if p == NP - 1:
                    o_piece(qbg)

            # ---- prologue compute: pair-0 K/Q on ring slots ----------------

            def ring_kq(groups):
                """groups: list of ('k'|'q', p, piece). One ring tile, one
                matmul group per bank, batched drains per contiguous run."""
                rt = ring.tile([128, 1024], F32, tag="ring")
                for g, (kind, p, piece) in enumerate(groups):
                    w = wkS if kind == "k" else wqS
                    for ct in range(CT):
                        nc.tensor.matmul(
                            rt[:, g * 512:(g + 1) * 512],
                            w[:, ct, p * 128:(p + 1) * 128],
                            xT[:, ct, piece * 512:(piece + 1) * 512],
                            start=(ct == 0), stop=(ct == CT - 1))
                g = 0
                while g < len(groups):
                    kind, p, piece = groups[g]
                    g2 = g
                    while (g2 + 1 < len(groups)
                           and groups[g2 + 1][0] == kind
                           and groups[g2 + 1][2] == groups[g2][2] + 1):
                        g2 += 1
                    dst = kf8[p % 2] if kind == "k" else qf8[p % 2]
                    bcol = 1 if kind == "k" else 0
                    drain_bias(
                        dst[:, 0, piece * 512:piece * 512 + (g2 - g + 1) * 512],
                        rt[:, g * 512:(g2 + 1) * 512], bcol, p,
                        (g2 - g + 1) * 512)
                    g = g2 + 1

            # PE p-state warm-up on zeros while x loads
            wup = wrk.tile([128, 512], F32, tag="wrk")
            for i in range(6):
                nc.tensor.matmul(wup[:], dz[:, :, 0:128], dz[:],
                                 start=True, stop=True, perf_mode=DR,
                                 tile_position=(0, 0))
            ring_kq([("k", 0, 0)])
            q_piece(0, 0)
            v2p_cur = new_v2p()

            # ---- main pipeline over 16 phases ----
            state = {"v2p": v2p_cur, "v2p_next": None, "pending": [],
                     "av_t": None, "o_queue": []}

            def o_drain():
                if state["o_queue"]:
                    o_piece(state["o_queue"].pop(0))

            def phase_background(ph):
                """Thunks to interleave into phase ph's chunk stream."""
                thunks = []
                h, half = ph // 2, ph % 2
                p = h // 2
                pend, state["pending"] = state["pending"], []
                thunks.extend(pend[:2])
                if state["av_t"] is not None:
                    av_t_prev = state["av_t"]
                    state["av_t"] = None
                    thunks.append(lambda ph=ph, t=av_t_prev:
                                  av_finish(ph - 1, t))
                thunks.extend(pend[2:])
                # pair-0 K/Q pieces MUST lead phase 0: its own chunks read
                # them piece-by-piece (program order is the only ordering)
                if ph == 0:
                    thunks.append(lambda: ring_kq([("q", 0, 1), ("k", 0, 1)]))
                    thunks.append(lambda: ring_kq([("k", 0, 2), ("k", 0, 3)]))
                    thunks.append(lambda: ring_kq([("k", 0, 4), ("k", 0, 5)]))
                    thunks.append(lambda: k_piece(0, 6))
                    thunks.append(lambda: k_piece(0, 7))
                # projection prep for pair p+1, spread over all 4 slots
                slot = ph % 4
                if p + 1 < NP:
                    if slot == 0:
                        for piece in range(3):
                            thunks.append(lambda p=p, piece=piece:
                                          k_piece(p + 1, piece))
                    elif slot == 1:
                        for piece in range(3, 6):
                            thunks.append(lambda p=p, piece=piece:
                                          k_piece(p + 1, piece))
                    elif slot == 2:
                        for piece in range(6, 8):
                            thunks.append(lambda p=p, piece=piece:
                                          k_piece(p + 1, piece))
                        thunks.append(lambda p=p: q_piece(p + 1, 0))
                    elif slot == 3:
                        thunks.append(lambda p=p: q_piece(p + 1, 1))
                if ph == 0:
                    for j in range(0, KT, 2):
                        thunks.append(lambda j=j: v_piece(state["v2p"], 0, j))
                if 4 <= ph <= 7:
                    if ph == 4:
                        def mkv():
                            state["v2p_next"] = new_v2p()
                        thunks.append(mkv)
                    for j in range((ph - 4) * 8, (ph - 4) * 8 + 8, 2):
                        thunks.append(lambda j=j: v_piece(state["v2p_next"],
                                                          1, j))
                if ph == 13:
                    for qt in range(4):
                        thunks.append(lambda qt=qt: o_partial(qt))
                if ph == 14:
                    for qt in range(4, 8):
                        thunks.append(lambda qt=qt: o_partial(qt))
                if ph == 15:
                    thunks.extend([o_drain] * 4)
                return thunks

            for ph in range(NPH):
                h, half = ph // 2, ph % 2
                if ph == 8:
                    state["v2p"] = state["v2p_next"]
                d0 = (h % 2) * 64
                kcur, qcur = kf8[h // 2 % 2], qf8[h // 2 % 2]
                bg = phase_background(ph)
                bgi = 0
                ncH = len(CHUNKS)
                for ci, (kt0, n) in enumerate(CHUNKS):
                    ring_t = ring.tile([128, 1024], F32, tag="ring")
                    for jj in range(n):
                        kt = kt0 + jj
                        nc.tensor.matmul(
                            ring_t[:, jj * 512:(jj + 1) * 512],
                            kcur[d0:d0 + 64, :, kt * 128:(kt + 1) * 128],
                            qcur[d0:d0 + 64, :, half * 512:(half + 1) * 512],
                            start=True, stop=True, perf_mode=DR,
                            tile_position=(d0, 0))
                    exp_chunk(ph, kt0, n, ring_t)
                    n_bg = (min(len(bg), (len(bg) * (ci + 1) + ncH - 1) // ncH)
                            - min(len(bg), (len(bg) * ci + ncH - 1) // ncH))
                    for _ in range(n_bg):
                        bg[bgi]()
                        bgi += 1
                assert bgi == len(bg)
                state["av_t"] = wrk.tile([128, 512], F32, tag="wrk",
                                         name="av_t")
                for qb in range(4):
                    for b in range(NB):
                        nc.tensor.matmul(
                            state["av_t"][:, qb * 128:qb * 128 + 65],
                            at[ph % 2][:, 2 * b:2 * b + 2,
                                       qb * 128:(qb + 1) * 128],
                            state["v2p"][:, b, :, (ph // 2) % 4, :],
                            start=(b == 0), stop=(b == NB - 1), perf_mode=DR)

            # ---- epilogue: AV(15) finish, per-qb pipelined with PE-mode
            # transposes (no 2.4us DMA-transpose latency at the tail) ----
            avE = state["av_t"]
            epR = [ring.tile([128, 1024], F32, tag="ring", name="epR0"),
                   ring.tile([128, 1024], F32, tag="ring", name="epR1")]
            for qb in range(4):
                qbg = 4 + qb
                rec1 = rpool.tile([128, 1], F32, tag="rec")
                nc.vector.reciprocal_approx_fast(
                    rec1[:], avE[:, qb * 128 + 64:qb * 128 + 65])
                nc.vector.tensor_scalar(
                    avn[1][:, qbg, 64:128],
                    avE[:, qb * 128:qb * 128 + 64],
                    rec1[:], None, MULT)
                trp = wrk.tile([128, 128], BF16, tag="wrk", name="trp")
                nc.tensor.transpose(trp[:], avn[1][:, qbg, :], eyeS[:])
                nc.vector.tensor_copy(acat[:, 3, qbg * 128:(qbg + 1) * 128],
                                      trp[:])
                o_piece(qbg, po=epR[qb // 2][:, (qb % 2) * 512:
                                             (qb % 2) * 512 + 512])

    nc.compile()
    return nc


def _prep_inputs(x, Wq, bq, Wk, bk, Wv, bv, Wo, bo):
    bf = ml_dtypes.bfloat16
    wqT = np.ascontiguousarray(Wq.T).astype(bf)
    wkT = np.ascontiguousarray(Wk.T).astype(bf)
    wvT = np.ascontiguousarray(Wv.T).astype(bf)
    woT = np.ascontiguousarray(Wo.T).astype(bf)
    bias = np.stack([
        bq.reshape(NP, 128).T,
        bk.reshape(NP, 128).T,
    ], axis=1).astype(np.float32)          # [128, 2, NP]
    bias = np.ascontiguousarray(bias)
    bof = np.ascontiguousarray(
        (bo.astype(np.float64) + bv.astype(np.float64) @ Wo.astype(np.float64).T)
        .reshape(1, C)).astype(np.float32)
    ones1 = np.ones((1, 128), np.float32)
    eye = np.eye(128, dtype=bf)
    in_maps = []
    for i in range(8):
        b, q0 = i // 4, (i % 4) * TQ
        xbT = np.ascontiguousarray(np.roll(x[b].T, -q0, axis=1)).astype(bf)
        in_maps.append({
            "xbT": xbT, "wqT": wqT, "wkT": wkT, "wvT": wvT, "woT": woT,
            "bias": bias, "bof": bof, "ones1": ones1, "eye": eye,
        })
    return in_maps


def kernel(x, Wq, bq, Wk, bk, Wv, bv, Wo, bo):
    x = np.asarray(x, np.float32)
    args = [np.asarray(a, np.float32) for a in
            (Wq, bq, Wk, bk, Wv, bv, Wo, bo)]
    if "nc" not in _cache:
        _cache["nc"] = _build()
    nc = _cache["nc"]
    in_maps = _prep_inputs(x, *args)
    res = run_bass_kernel_spmd(nc, in_maps, list(range(8)))
    outf = np.empty((B, T, C), np.float32)
    for i in range(8):
        b, q0 = i // 4, (i % 4) * TQ
        outf[b, q0:q0 + TQ, :] = res.results[i]["out"].astype(np.float32)
    return outf
